# revision 29
# baseline (speedup 1.0000x reference)
"""Two-layer GAT (8-head + 1-head) Trainium2 Bass kernel, 8-way node-sharded.

Strategy (per core c, owning row block I_c of R = N/8 nodes):
  * Scores live in TRANSPOSED layout [j (partition), i (free)] so the
    aggregation matmul out^T[o, i] = sum_j h[j, o] * P[j, i] contracts over
    the partition dim naturally (lhsT = h rows, rhs = P^T tile).
  * Softmax factorization: exp(lrelu(fs[i] + fd[j])) =
      exp(.2 fs[i]) * max(exp(.8 fs[i] + fd[j]), exp(.2 fd[j]))
    and the exp(.2 fs[i]) factor is constant along the softmax axis j, so it
    cancels in P/Z.
  * The adjacency mask folds into the scores on the TensorEngine: the score
    PSUM accumulates BIG*adj (lhsT = BIG*I, rhs = adjT chunk) and the exp
    bias subtracts BIG, so exp(u + BIG*(adj-1)) ~ 0 off-edges.  Per score
    tile the elementwise work is then ONE ScalarE exp plus ONE fused DVE op:
      P = (adjT * vd[j]) max exp(u'')     (scalar_tensor_tensor)
    with a GpSimd share.  This also keeps the PE stream dense (HAM-warm).
  * u[j, i] = .8 fs[i] + fd[j] comes from a rank-3 matmul (lhsT rows =
    [ones, fd_h0, fd_h1] at a 32-aligned partition base, rhs = packed
    .8*fs row + block-diag ones rows), two heads per PSUM tile.
  * Z_i = sum_j P[j, i] via col-tiled ones matmuls (two heads in disjoint
    32-partition blocks of one PSUM tile, issued adjacent for overlap).
  * f vectors fall out of the feature transform for free by running it with
    the folded vector w = W @ a as the STATIONARY operand and the x^T tile
    moving: the result lands pre-transposed at exactly the partitions the
    score matmuls need.
  * x is uploaded pre-arranged to the exact SBUF layout (one contiguous DMA
    slice per chunk); adjacency is uploaded uint8 and widened on-device.
  * Layer-2 h2 = h1 @ W2 partials accumulate after each head-pair pass; one
    AllGather distributes h2 ([N, 18]).  Layer-2 aggregation and Z2 come
    from ONE matmul with stationary [h2 | 0 | ones] (Z2 at partition 32).
"""

import sys

sys.path.insert(0, "/opt/trn_rl_repo")

import numpy as np
import ml_dtypes

N = 4096
F_IN = 512
H1 = 8
D1 = 128
F1 = 1024          # H1 * D1
D2 = 16
NCORES = 8
R = N // NCORES    # rows (nodes) per core
NCH = N // 128     # j-chunks of 128
NFC = F_IN // 128  # f chunks
ALPHA = 0.2
BIG = 30.0         # mask shift: exp(-BIG + u) ~ 0 for off-edges

GPS_EVERY = 4      # gpsimd takes the mask multiply on every 4th sub-step
PEMASK_EVERY = 4   # chunks where the PE accumulates BIG*adj (fused DVE op)

_BUILD_CACHE = {}


def _build_nc():
    import concourse.bacc as bacc
    import concourse.tile as tile
    import concourse.mybir as mybir

    FP32 = mybir.dt.float32
    BF16 = mybir.dt.bfloat16
    U8 = mybir.dt.uint8
    AF = mybir.ActivationFunctionType
    OP = mybir.AluOpType
    AX = mybir.AxisListType

    nc = bacc.Bacc(num_devices=NCORES)

    # ---- I/O -------------------------------------------------------------
    # xTa[p, (jt, fc, o)] = x[jt*128+o, fc*128+p]  (pre-arranged SBUF layout)
    xTa_d = nc.dram_tensor("xTa", [128, NCH * NFC * 128], BF16,
                           kind="ExternalInput")
    # xToa: same layout for the core's own 4 chunks
    xToa_d = nc.dram_tensor("xToa", [128, 4 * NFC * 128], BF16,
                            kind="ExternalInput")
    W1_d = nc.dram_tensor("W1f", [F_IN, F1], BF16, kind="ExternalInput")
    wsd1_d = nc.dram_tensor("wsd1", [F_IN, 128], BF16, kind="ExternalInput")
    wso1_d = nc.dram_tensor("wso1", [F_IN, 256], BF16, kind="ExternalInput")
    wdn_d = nc.dram_tensor("wdn", [F_IN, 8], BF16, kind="ExternalInput")
    adjU8_d = nc.dram_tensor("adjU8", [N, R], U8, kind="ExternalInput")
    W2a_d = nc.dram_tensor("W2a", [F1, 18], BF16, kind="ExternalInput")
    out_d = nc.dram_tensor("out", [R, D2], FP32, kind="ExternalOutput")

    def dma_chunked(dst_tile, src_ap, inner, batch=False):
        # [C*128, inner] DRAM -> [128, C*inner] SBUF (chunk-major free dim)
        if batch:
            nc.sync.dma_start(
                dst_tile[:].rearrange("p (c o) -> p c o", o=inner),
                src_ap.rearrange("(c p) o -> p c o", p=128),
            )
            return
        nchunks = src_ap.shape[0] // 128
        for cc in range(nchunks):
            nc.sync.dma_start(
                dst_tile[:, cc * inner:(cc + 1) * inner],
                src_ap[cc * 128:(cc + 1) * 128, :],
            )

    with tile.TileContext(nc) as tc:
        with (
            tc.tile_pool(name="const", bufs=1) as cpool,
            tc.tile_pool(name="dram", bufs=1, space="DRAM") as dpool,
        ):
            # ---- resident SBUF tensors (weights first: phase 1 needs them)
            W1_sb = cpool.tile([128, NFC * F1], BF16, tag="W1")
            dma_chunked(W1_sb, W1_d[:], F1)
            wsd1_sb = cpool.tile([128, NFC * 128], BF16, tag="wsd1")
            dma_chunked(wsd1_sb, wsd1_d[:], 128)
            wso1_sb = cpool.tile([128, NFC * 256], BF16, tag="wso1")
            dma_chunked(wso1_sb, wso1_d[:], 256)
            wdn_sb = cpool.tile([128, NFC * 8], BF16, tag="wdn")
            dma_chunked(wdn_sb, wdn_d[:], 8)
            W2a_sb = cpool.tile([128, (F1 // 128) * 18], BF16, tag="W2a")
            dma_chunked(W2a_sb, W2a_d[:], 18)

            adjT_sb = cpool.tile([128, NCH * R], BF16, tag="adjT")

            # on-device constants
            onesb_sb = cpool.tile([128, 128], BF16, tag="onesb")
            nc.vector.memset(onesb_sb[:], 1.0)
            idb_sb = cpool.tile([128, 128], BF16, tag="idb")
            nc.gpsimd.affine_select(
                idb_sb[:], onesb_sb[:], [[-1, 128]],
                mybir.AluOpType.is_equal, 0.0,
                base=0, channel_multiplier=1,
            )
            idBIG_sb = cpool.tile([128, 128], BF16, tag="idBIG")
            nc.vector.tensor_scalar_mul(idBIG_sb[:], idb_sb[:], BIG)
            idf_sb = cpool.tile([16, 16], FP32, tag="idf")
            nc.gpsimd.memset(idf_sb[:], 1.0)
            nc.gpsimd.affine_select(
                idf_sb[:], idf_sb[:], [[-1, 16]],
                mybir.AluOpType.is_equal, 0.0,
                base=0, channel_multiplier=1,
            )
            bigneg = cpool.tile([128, 1], FP32, tag="bigneg")
            nc.vector.memset(bigneg[:], -BIG)

            h_sb = cpool.tile([128, NCH * F1], BF16, tag="h")
            fdT = cpool.tile([128, N], BF16, tag="fdT")
            srcpat = cpool.tile([128, 2 * R], BF16, tag="srcpat")
            vd_sb = cpool.tile([128, NCH * 8], FP32, tag="vd")
            h1T = cpool.tile([128, H1 * R], BF16, tag="h1T")
            h2acc = cpool.tile([128, (R // 128) * 18], FP32, tag="h2acc")
            h2all_sb = cpool.tile([128, NCH * 18], BF16, tag="h2all")
            h2e_sb = cpool.tile([128, NCH * 33], BF16, tag="h2e")
            f2T = cpool.tile([2, N], BF16, tag="f2T")
            srcpat2 = cpool.tile([2, R], BF16, tag="srcpat2")
            fsd2 = cpool.tile([2, R], BF16, tag="fsd2")
            vd2_sb = cpool.tile([128, NCH], FP32, tag="vd2")

            h2loc = dpool.tile([R, 18], BF16, tag="h2loc")
            h2all_d = dpool.tile([N, 18], BF16, tag="h2all",
                                 addr_space="Shared")

            # ones row staged in DRAM (engine writes must be 32-part aligned;
            # DMA writes are not restricted)
            onesN_sb = cpool.tile([1, 1024], BF16, tag="onesN")
            nc.vector.memset(onesN_sb[:], 1.0)
            onesN_d = dpool.tile([1, 1024], BF16, tag="onesNd")
            nc.sync.dma_start(onesN_d[:], onesN_sb[:])

            # tiny warm-up collective: pays CC-core init/skew cost early,
            # overlapped with input DMA + phase 1 (nothing depends on it)
            wup_in = dpool.tile([1, 64], BF16, tag="wupi")
            wup_out = dpool.tile([NCORES, 64], BF16, tag="wupo",
                                 addr_space="Shared")
            nc.sync.dma_start(wup_in[:], onesN_d[:, 0:64])
            nc.gpsimd.collective_compute(
                "AllGather",
                OP.bypass,
                replica_groups=[list(range(NCORES))],
                ins=[wup_in[:].opt()],
                outs=[wup_out[:].opt()],
            )

            # Z-selector: ones in cols 32k..32k+32 put head k's Z at
            # partitions 32k..32k+32 (plain matmuls; no col-tiling)
            zsel_sb = cpool.tile([128, 256], BF16, tag="zsel")
            nc.vector.memset(zsel_sb[:], 0.0)
            nc.vector.memset(zsel_sb[:, 0:32], 1.0)
            nc.vector.memset(zsel_sb[:, 128 + 32:128 + 64], 1.0)

            # srcpat block-diag ones rows (rows 32p+1/+2); zero first
            nc.vector.memset(srcpat[:], 0.0)
            for p in range(4):
                nc.sync.dma_start(
                    srcpat[32 * p + 1:32 * p + 2, 0:R], onesN_d[:, 0:R]
                )
                nc.sync.dma_start(
                    srcpat[32 * p + 2:32 * p + 3, R:2 * R], onesN_d[:, 0:R]
                )

            # =============================================================
            # Phase 1: h = x @ W1 (all nodes); f_dst^T rows via stationary-w
            # matmuls; v_dst = exp(.2 f_dst) columns; own-block .8*f_src^T.
            # x staged in quarters (8 chunks each) to bound SBUF.
            # =============================================================
            with (
                tc.tile_pool(name="xq", bufs=2) as xqpool,
                tc.tile_pool(name="xo", bufs=1) as xopool,
                tc.tile_pool(name="adj8", bufs=1) as adj8pool,
                tc.tile_pool(name="ph", bufs=2, space="PSUM") as php,
                tc.tile_pool(name="pf", bufs=2, space="PSUM") as pfp,
            ):
                # own block .8*f_src^T rows, written at partitions 32p
                xo = xopool.tile([128, 4 * NFC * 128], BF16, tag="xo")
                nc.sync.dma_start(xo[:], xToa_d[:])
                for jt2 in range(R // 128):
                    for k in range(2):
                        pfo = pfp.tile([128, 128], FP32, tag="pft")
                        for fc in range(NFC):
                            nc.tensor.matmul(
                                pfo[:],
                                wso1_sb[:, fc * 256 + 128 * k:
                                        fc * 256 + 128 * (k + 1)],
                                xo[:, (jt2 * NFC + fc) * 128:
                                   (jt2 * NFC + fc + 1) * 128],
                                start=fc == 0, stop=fc == NFC - 1,
                            )
                        for p in range(4):
                            nc.vector.tensor_copy(
                                srcpat[32 * p:32 * p + 1,
                                       k * R + jt2 * 128:
                                       k * R + (jt2 + 1) * 128],
                                pfo[32 * p:32 * p + 1, :],
                            )
                adjU8_sb = adj8pool.tile([128, NCH * R], U8, tag="a8")
                for q in range(4):
                    xq = xqpool.tile([128, 8 * NFC * 128], BF16, tag="xq")
                    nc.sync.dma_start(
                        xq[:], xTa_d[:, q * 8 * NFC * 128:
                                     (q + 1) * 8 * NFC * 128]
                    )
                    # adjacency quarter: DMA + u8->bf16 widening (DVE slack)
                    sl = slice(q * 8 * R, (q + 1) * 8 * R)
                    for cc in range(8):
                        ch = q * 8 + cc
                        nc.sync.dma_start(
                            adjU8_sb[:, ch * R:(ch + 1) * R],
                            adjU8_d[ch * 128:(ch + 1) * 128, :],
                        )
                    nc.vector.tensor_copy(adjT_sb[:, sl], adjU8_sb[:, sl])
                    for jl in range(8):
                        jt = q * 8 + jl
                        ph = php.tile([128, F1], FP32, tag="ph")
                        pft = pfp.tile([128, 128], FP32, tag="pft")
                        pfn = pfp.tile([128, 8], FP32, tag="pfn")
                        for fc in range(NFC):
                            lhs = xq[:, (jl * NFC + fc) * 128:
                                     (jl * NFC + fc + 1) * 128]
                            st = fc == 0
                            sp = fc == NFC - 1
                            nc.tensor.matmul(
                                ph[:, 0:512], lhs,
                                W1_sb[:, fc * F1:fc * F1 + 512],
                                start=st, stop=sp,
                            )
                            nc.tensor.matmul(
                                ph[:, 512:F1], lhs,
                                W1_sb[:, fc * F1 + 512:(fc + 1) * F1],
                                start=st, stop=sp,
                            )
                            nc.tensor.matmul(
                                pfn[:], lhs, wdn_sb[:, fc * 8:(fc + 1) * 8],
                                start=st, stop=sp,
                            )
                            nc.tensor.matmul(
                                pft[:], wsd1_sb[:, fc * 128:(fc + 1) * 128],
                                lhs, start=st, stop=sp,
                            )
                        # evacuate h (split DVE / ACT), f_dst^T, v_dst cols
                        nc.vector.tensor_copy(
                            h_sb[:, jt * F1:jt * F1 + 512], ph[:, 0:512]
                        )
                        nc.scalar.activation(
                            h_sb[:, jt * F1 + 512:(jt + 1) * F1],
                            ph[:, 512:F1], AF.Copy,
                        )
                        nc.vector.tensor_copy(
                            fdT[:, jt * 128:(jt + 1) * 128], pft[:]
                        )
                        nc.scalar.activation(
                            vd_sb[:, jt * 8:(jt + 1) * 8], pfn[:],
                            AF.Exp, scale=ALPHA,
                        )
                # ones rows of fdT (after the copies; WAW-ordered)
                for p in range(4):
                    nc.vector.memset(fdT[32 * p:32 * p + 1, :], 1.0)


            # =============================================================
            # Phase 2: layer-1 attention, 2 heads (one pair) per pass
            # =============================================================
            with (
                tc.tile_pool(name="acc", bufs=1, space="PSUM") as acc,
                tc.tile_pool(name="pe", bufs=5, space="PSUM") as epool,
                tc.tile_pool(name="sc", bufs=6) as spool,
                tc.tile_pool(name="pt", bufs=4) as ptpool,
                tc.tile_pool(name="nrm", bufs=1) as npool,
            ):
                for hp in range(4):
                    p = hp  # pair index; heads (2p, 2p+1)
                    po = [
                        acc.tile([128, R], FP32, tag=f"o{k}", name=f"po{k}")
                        for k in range(2)
                    ]
                    pz = acc.tile([128, R], FP32, tag="z")
                    for c in range(NCH):
                        adj_c = adjT_sb[:, c * R:(c + 1) * R]
                        lhs_e = fdT[32 * p:32 * p + 3, c * 128:(c + 1) * 128]
                        pemask = PEMASK_EVERY and c % PEMASK_EVERY == 0
                        for k in range(2):
                            g = 2 * p + k
                            vd_ap = vd_sb[:, c * 8 + g:c * 8 + g + 1]
                            pek = epool.tile([128, R], FP32, tag="pe")
                            nc.tensor.matmul(
                                pek[:], lhs_e,
                                srcpat[32 * p:32 * p + 3, k * R:(k + 1) * R],
                                start=True, stop=not pemask,
                                tile_position=(32 * p, 0),
                            )
                            if pemask:
                                nc.tensor.matmul(
                                    pek[:], idBIG_sb[:], adj_c,
                                    start=False, stop=True,
                                )
                            t1 = spool.tile([128, R], BF16, tag="t1")
                            if pemask:
                                nc.scalar.activation(
                                    t1[:], pek[:], AF.Exp, bias=bigneg[:, 0:1]
                                )
                            else:
                                nc.scalar.activation(t1[:], pek[:], AF.Exp)
                            pt = ptpool.tile([128, R], BF16, tag="pt")
                            if pemask:
                                nc.vector.scalar_tensor_tensor(
                                    pt[:], adj_c, vd_ap, t1[:],
                                    op0=OP.mult, op1=OP.max,
                                )
                            else:
                                q = spool.tile([128, R], BF16, tag="q")
                                nc.vector.tensor_scalar_max(q[:], t1[:], vd_ap)
                                eng = (nc.gpsimd
                                       if (2 * c + k) % GPS_EVERY == 0
                                       else nc.vector)
                                eng.tensor_mul(pt[:], q[:], adj_c)
                            nc.tensor.matmul(
                                po[k][:],
                                h_sb[:, c * F1 + g * D1:c * F1 + (g + 1) * D1],
                                pt[:],
                                start=c == 0, stop=c == NCH - 1,
                            )
                            nc.tensor.matmul(
                                pz[:],
                                zsel_sb[:, k * 128:(k + 1) * 128],
                                pt[:],
                                start=(c == 0 and k == 0),
                                stop=(c == NCH - 1 and k == 1),
                            )
                    # normalize + ELU -> h1^T (bf16).  Evacuate po/pz to
                    # SBUF immediately so the accumulator banks free up for
                    # the next pair (PE keeps streaming, stays HAM-warm).
                    zinv = npool.tile([64, R], FP32, tag="zinv")
                    nc.vector.reciprocal_approx_fast(zinv[:], pz[0:64, :])
                    zinvb = npool.tile([64, R], BF16, tag="zinvb")
                    nc.vector.tensor_copy(zinvb[:], zinv[:])
                    po_sb = [npool.tile([128, R], FP32, tag=f"posb{k}",
                                        name=f"po_sb{k}")
                             for k in range(2)]
                    nc.vector.tensor_copy(po_sb[0][:], po[0][:])
                    nc.scalar.activation(po_sb[1][:], po[1][:], AF.Copy)
                    for k in range(2):
                        g = 2 * p + k
                        zb_ps = epool.tile([128, R], FP32, tag="pe")
                        nc.tensor.matmul(
                            zb_ps[:],
                            onesb_sb[32 * k:32 * k + 1, :],
                            zinvb[32 * k:32 * k + 1, :],
                            start=True, stop=True,
                            tile_position=(32 * k, 0),
                        )
                        zb_sb = npool.tile([128, R], FP32, tag="zbs")
                        nc.vector.tensor_copy(zb_sb[:], zb_ps[:])
                        pre = npool.tile([128, R], FP32, tag="pre")
                        nc.vector.tensor_mul(pre[:], po_sb[k][:], zb_sb[:])
                        r = npool.tile([128, R], FP32, tag="r")
                        nc.scalar.activation(r[:], pre[:], AF.Relu, scale=-1.0)
                        t = npool.tile([128, R], FP32, tag="t")
                        nc.scalar.activation(t[:], r[:], AF.Exp, scale=-1.0)
                        u = npool.tile([128, R], FP32, tag="u")
                        nc.vector.tensor_scalar_add(u[:], t[:], -1.0)
                        nc.vector.tensor_max(
                            h1T[:, g * R:(g + 1) * R], pre[:], u[:]
                        )
                    # h2 partial: accumulate this pair's 2 feature blocks
                    for jt2 in range(R // 128):
                        ph2f = epool.tile([128, R], FP32, tag="pe")
                        ph2 = ph2f[:, 0:18]
                        for k in range(2):
                            g = 2 * p + k
                            nc.tensor.matmul(
                                ph2,
                                h1T[:, g * R + jt2 * 128:
                                    g * R + (jt2 + 1) * 128],
                                W2a_sb[:, g * 18:(g + 1) * 18],
                                start=k == 0, stop=k == 1,
                            )
                        dst = h2acc[:, jt2 * 18:(jt2 + 1) * 18]
                        if p == 0:
                            nc.vector.tensor_copy(dst, ph2)
                        else:
                            nc.vector.tensor_add(dst, dst, ph2)

            # =============================================================
            # Phase 3: AllGather h2 ([N, 18]); rebuild f2 rows
            # =============================================================
            with (
                tc.tile_pool(name="p2t", bufs=2, space="PSUM") as p2tp,
                tc.tile_pool(name="h2s", bufs=2) as h2p,
            ):
                for jt2 in range(R // 128):
                    h2t = h2p.tile([128, 18], BF16, tag="h2t")
                    nc.vector.tensor_copy(
                        h2t[:], h2acc[:, jt2 * 18:(jt2 + 1) * 18]
                    )
                    nc.sync.dma_start(
                        h2loc[jt2 * 128:(jt2 + 1) * 128, :], h2t[:]
                    )
                    # transpose cols 16:18 -> [fs2 row; fd2 row] (local)
                    ps2 = p2tp.tile([2, 128], BF16, tag="ps2")
                    nc.tensor.transpose(ps2[:], h2t[:, 16:18], idb_sb[:])
                    nc.vector.tensor_copy(
                        fsd2[0:2, jt2 * 128:(jt2 + 1) * 128], ps2[0:2, :]
                    )
                # zero-fill h2e while the gather is in flight
                nc.vector.memset(h2e_sb[:], 0.0)
                nc.gpsimd.collective_compute(
                    "AllGather",
                    OP.bypass,
                    replica_groups=[list(range(NCORES))],
                    ins=[h2loc[:].opt()],
                    outs=[h2all_d[:].opt()],
                )
                dma_chunked(h2all_sb, h2all_d[:], 18, batch=True)
                for q4 in range(4):
                    nc.sync.dma_start(
                        f2T[1:2, q4 * 1024:(q4 + 1) * 1024], onesN_d[:]
                    )
                # srcpat2 rows: [ones; .8*fs2 (local, pre-scaled via W2a)]
                nc.vector.memset(srcpat2[0:1, :], 1.0)
                nc.sync.dma_start(srcpat2[1:2, :], fsd2[0:1, :])
                # layer-2 exp(.2 f_dst2) columns
                nc.scalar.activation(
                    vd2_sb[:, 0:NCH], h2all_sb[:, 17:NCH * 18:18],
                    AF.Exp, scale=ALPHA,
                )
                # h2e = [h2 (16) | zeros (16) | ones] per chunk: the ones
                # column puts Z2 at PSUM partition 32 (32-aligned for reads)
                nc.vector.tensor_copy(
                    h2e_sb[:].rearrange("p (c o) -> p c o", o=33)[:, :, 0:16],
                    h2all_sb[:].rearrange("p (c o) -> p c o", o=18)[:, :, 0:16],
                )
                nc.vector.memset(
                    h2e_sb[:].rearrange("p (c o) -> p c o", o=33)[:, :, 32:33],
                    1.0,
                )

            # =============================================================
            # Phase 4: layer-2 attention + ELU + log_softmax
            # =============================================================
            with (
                tc.tile_pool(name="acc2", bufs=1, space="PSUM") as acc2,
                tc.tile_pool(name="pe2", bufs=3, space="PSUM") as e2pool,
                tc.tile_pool(name="sc2", bufs=6) as spool2,
                tc.tile_pool(name="fin", bufs=2) as fpool,
                tc.tile_pool(name="pfin", bufs=1, space="PSUM") as pfp2,
                tc.tile_pool(name="pc", bufs=2, space="PSUM") as pcp,
            ):
                po2 = acc2.tile([64, R], FP32, tag="o2")
                for c in range(NCH):
                    adj_c = adjT_sb[:, c * R:(c + 1) * R]
                    # f2T fd2-row chunk from h2all col 17 (interleaved so
                    # chunk 0's scores don't wait on all 32 transposes)
                    pcol = pcp.tile([1, 128], BF16, tag="pcol")
                    nc.tensor.transpose(
                        pcol[:], h2all_sb[:, c * 18 + 17:c * 18 + 18],
                        idb_sb[:],
                    )
                    nc.vector.tensor_copy(
                        f2T[0:1, c * 128:(c + 1) * 128], pcol[:]
                    )
                    pe2 = e2pool.tile([128, R], FP32, tag="pe2")
                    nc.tensor.matmul(
                        pe2[:], f2T[:, c * 128:(c + 1) * 128], srcpat2[:],
                        start=True, stop=False,
                    )
                    nc.tensor.matmul(
                        pe2[:], idBIG_sb[:], adj_c,
                        start=False, stop=True,
                    )
                    t1 = spool2.tile([128, R], BF16, tag="t1b")
                    nc.scalar.activation(
                        t1[:], pe2[:], AF.Exp, bias=bigneg[:, 0:1]
                    )
                    pt2 = spool2.tile([128, R], BF16, tag="pt2")
                    nc.vector.scalar_tensor_tensor(
                        pt2[:], adj_c, vd2_sb[:, c:c + 1], t1[:],
                        op0=OP.mult, op1=OP.max,
                    )
                    nc.tensor.matmul(
                        po2[0:33, :], h2e_sb[:, c * 33:(c + 1) * 33], pt2[:],
                        start=c == 0, stop=c == NCH - 1,
                    )
                # approx-reciprocal misreads PSUM at a non-zero partition
                # base; stage the Z2 row to SBUF partition 0 first
                z2sb = fpool.tile([1, R], FP32, tag="z2sb")
                nc.vector.tensor_copy(z2sb[:], po2[32:33, :])
                zinv2 = fpool.tile([1, R], FP32, tag="zinv2")
                nc.vector.reciprocal_approx_fast(zinv2[:], z2sb[:])
                zinv2b = fpool.tile([1, R], BF16, tag="zinv2b")
                nc.vector.tensor_copy(zinv2b[:], zinv2[:])
                zb2_ps = pfp2.tile([16, R], FP32, tag="zb2")
                nc.tensor.matmul(
                    zb2_ps[:], onesb_sb[0:1, 0:16], zinv2b[:],
                    start=True, stop=True,
                )
                zb2 = fpool.tile([16, R], FP32, tag="zb2s")
                nc.vector.tensor_copy(zb2[:], zb2_ps[:])
                pre2 = fpool.tile([16, R], FP32, tag="pre2")
                nc.vector.tensor_mul(pre2[:], po2[0:16, :], zb2[:])
                r2 = fpool.tile([16, R], FP32, tag="r2")
                nc.scalar.activation(r2[:], pre2[:], AF.Relu, scale=-1.0)
                t2e = fpool.tile([16, R], FP32, tag="t2e")
                nc.scalar.activation(t2e[:], r2[:], AF.Exp, scale=-1.0)
                u2 = fpool.tile([16, R], FP32, tag="u2")
                nc.vector.tensor_scalar_add(u2[:], t2e[:], -1.0)
                elu2 = fpool.tile([16, R], FP32, tag="elu2")
                nc.vector.tensor_max(elu2[:], pre2[:], u2[:])
                # transpose to natural [i, o2] then log_softmax over free
                # dim; exps batched before one Ln (fewer ACT table switches)
                pns, nmxs = [], []
                s_all = fpool.tile([128, 4], FP32, tag="s_all")
                for it in range(R // 128):
                    pn = fpool.tile([128, 16], FP32, tag=f"pn{it}",
                                    name=f"pn{it}")
                    pnp = pfp2.tile([128, 16], FP32, tag="pn")
                    nc.tensor.transpose(
                        pnp[:], elu2[:, it * 128:(it + 1) * 128],
                        idf_sb[:],
                    )
                    nc.vector.tensor_copy(pn[:], pnp[:])
                    nmx = fpool.tile([128, 1], FP32, tag=f"nmx{it}",
                                     name=f"nmx{it}")
                    nc.vector.tensor_reduce(
                        nmx[:], pn[:], AX.X, OP.max, negate=True
                    )
                    ex = fpool.tile([128, 16], FP32, tag="ex")
                    nc.scalar.activation(
                        ex[:], pn[:], AF.Exp, bias=nmx[:, 0:1],
                        accum_out=s_all[:, it:it + 1],
                    )
                    pns.append(pn); nmxs.append(nmx)
                lg = fpool.tile([128, 4], FP32, tag="lg")
                nc.scalar.activation(lg[:], s_all[:], AF.Ln)
                for it in range(R // 128):
                    fin = fpool.tile([128, 16], FP32, tag="fin")
                    nc.vector.tensor_scalar(
                        fin[:], pns[it][:], nmxs[it][:, 0:1], lg[:, it:it + 1],
                        op0=OP.add, op1=OP.subtract,
                    )
                    nc.sync.dma_start(out_d[it * 128:(it + 1) * 128, :], fin[:])

    nc.compile()
    return nc


def _get_nc():
    if "nc" not in _BUILD_CACHE:
        _BUILD_CACHE["nc"] = _build_nc()
    return _BUILD_CACHE["nc"]


def _prep_inputs(x, adj, W1, a_src1, a_dst1, W2, a_src2, a_dst2):
    bf16 = ml_dtypes.bfloat16
    f32 = np.float32
    x = np.asarray(x, f32)
    adj = np.asarray(adj, f32)
    W1 = np.asarray(W1, f32)
    W2 = np.asarray(W2, f32)
    a_src1 = np.asarray(a_src1, f32)
    a_dst1 = np.asarray(a_dst1, f32)
    a_src2 = np.asarray(a_src2, f32)
    a_dst2 = np.asarray(a_dst2, f32)

    W1f = np.ascontiguousarray(W1.reshape(F_IN, F1))
    # folded score vectors: f_src[h] = x @ (W1[:,h,:] @ a_src1[h])
    wsrc = np.stack([W1[:, h, :] @ a_src1[h] for h in range(H1)], axis=1)
    wdst = np.stack([W1[:, h, :] @ a_dst1[h] for h in range(H1)], axis=1)
    # pair p lives at partitions 32p..32p+2: [ones, fd_2p, fd_2p+1]
    wsd1 = np.zeros((F_IN, 128), f32)
    for p in range(4):
        wsd1[:, 32 * p + 1] = wdst[:, 2 * p]
        wsd1[:, 32 * p + 2] = wdst[:, 2 * p + 1]
    # .8*f_src columns placed so the transform emits rows at partition 32p
    wso1 = np.zeros((F_IN, 256), f32)
    for k in range(2):
        for p in range(4):
            wso1[:, 128 * k + 32 * p] = 0.8 * wsrc[:, 2 * p + k]
    W2f = np.ascontiguousarray(W2.reshape(F1, D2))
    W2a = np.zeros((F1, 18), f32)
    W2a[:, :D2] = W2f
    W2a[:, 16] = 0.8 * (W2f @ a_src2[0])
    W2a[:, 17] = W2f @ a_dst2[0]

    adjT_u8 = (adj.T > 0).astype(np.uint8)
    xb = x.astype(bf16)
    # xTa[p, (jt, fc, o)] = x[jt*128+o, fc*128+p]
    xTa = np.ascontiguousarray(
        xb.reshape(NCH, 128, NFC, 128).transpose(3, 0, 2, 1)
    ).reshape(128, NCH * NFC * 128)

    shared = {
        "W1f": W1f.astype(bf16),
        "wsd1": wsd1.astype(bf16),
        "wso1": wso1.astype(bf16),
        "wdn": wdst.astype(bf16),
        "W2a": W2a.astype(bf16),
        "xTa": xTa,
    }
    in_maps = []
    for c in range(NCORES):
        blkslice = slice(c * R, (c + 1) * R)
        m = dict(shared)
        m["adjU8"] = np.ascontiguousarray(adjT_u8[:, blkslice])
        m["xToa"] = np.ascontiguousarray(
            xTa[:, c * 4 * NFC * 128:(c + 1) * 4 * NFC * 128]
        )
        in_maps.append(m)
    return in_maps


def kernel(x, adj, W1, a_src1, a_dst1, W2, a_src2, a_dst2, _trace=False):
    from concourse.bass_utils import run_bass_kernel_spmd

    nc = _get_nc()
    in_maps = _prep_inputs(x, adj, W1, a_src1, a_dst1, W2, a_src2, a_dst2)
    res = run_bass_kernel_spmd(nc, in_maps, list(range(NCORES)), trace=_trace)
    out = np.concatenate(
        [np.asarray(res.results[c]["out"]) for c in range(NCORES)], axis=0
    )
    kernel.last_results = res
    return out.astype(np.float32)


# revision 31
# speedup vs baseline: 1.1285x; 1.1285x over previous
"""Two-layer GAT (8-head + 1-head) Trainium2 Bass kernel, 8-way node-sharded.

Strategy (per core c, owning row block I_c of R = N/8 nodes):
  * Scores live in TRANSPOSED layout [j (partition), i (free)] so the
    aggregation matmul out^T[o, i] = sum_j h[j, o] * P[j, i] contracts over
    the partition dim naturally (lhsT = h rows, rhs = P^T tile).
  * Softmax factorization: exp(lrelu(fs[i] + fd[j])) =
      exp(.2 fs[i]) * max(exp(.8 fs[i] + fd[j]), exp(.2 fd[j]))
    and the exp(.2 fs[i]) factor is constant along the softmax axis j, so it
    cancels in P/Z.
  * The adjacency mask folds into the scores on the TensorEngine: the score
    PSUM accumulates BIG*adj (lhsT = BIG*I, rhs = adjT chunk) and the exp
    bias subtracts BIG, so exp(u + BIG*(adj-1)) ~ 0 off-edges.  Per score
    tile the elementwise work is then ONE ScalarE exp plus ONE fused DVE op:
      P = (adjT * vd[j]) max exp(u'')     (scalar_tensor_tensor)
    with a GpSimd share.  This also keeps the PE stream dense (HAM-warm).
  * u[j, i] = .8 fs[i] + fd[j] comes from a rank-3 matmul (lhsT rows =
    [ones, fd_h0, fd_h1] at a 32-aligned partition base, rhs = packed
    .8*fs row + block-diag ones rows), two heads per PSUM tile.
  * Z_i = sum_j P[j, i] via col-tiled ones matmuls (two heads in disjoint
    32-partition blocks of one PSUM tile, issued adjacent for overlap).
  * f vectors fall out of the feature transform for free by running it with
    the folded vector w = W @ a as the STATIONARY operand and the x^T tile
    moving: the result lands pre-transposed at exactly the partitions the
    score matmuls need.
  * x is uploaded pre-arranged to the exact SBUF layout (one contiguous DMA
    slice per chunk); adjacency is uploaded uint8 and widened on-device.
  * Layer-2 h2 = h1 @ W2 partials accumulate after each head-pair pass; one
    AllGather distributes h2 ([N, 18]).  Layer-2 aggregation and Z2 come
    from ONE matmul with stationary [h2 | 0 | ones] (Z2 at partition 32).
"""

import sys

sys.path.insert(0, "/opt/trn_rl_repo")

import numpy as np
import ml_dtypes

N = 4096
F_IN = 512
H1 = 8
D1 = 128
F1 = 1024          # H1 * D1
D2 = 16
NCORES = 8
R = N // NCORES    # rows (nodes) per core
NCH = N // 128     # j-chunks of 128
NFC = F_IN // 128  # f chunks
ALPHA = 0.2
BIG = 30.0         # mask shift: exp(-BIG + u) ~ 0 for off-edges

GPS_EVERY = 4      # gpsimd takes the mask multiply on every 4th sub-step
PEMASK_EVERY = 4   # chunks where the PE accumulates BIG*adj (fused DVE op)

_BUILD_CACHE = {}


def _build_nc():
    import concourse.bacc as bacc
    import concourse.tile as tile
    import concourse.mybir as mybir

    FP32 = mybir.dt.float32
    BF16 = mybir.dt.bfloat16
    U8 = mybir.dt.uint8
    AF = mybir.ActivationFunctionType
    OP = mybir.AluOpType
    AX = mybir.AxisListType

    nc = bacc.Bacc(num_devices=NCORES)

    # ---- I/O -------------------------------------------------------------
    # xTa[p, (jt, fc, o)] = x[jt*128+o, fc*128+p]  (pre-arranged SBUF layout)
    xTa_d = nc.dram_tensor("xTa", [128, NCH * NFC * 128], BF16,
                           kind="ExternalInput")
    # xToa: same layout for the core's own 4 chunks
    xToa_d = nc.dram_tensor("xToa", [128, 4 * NFC * 128], BF16,
                            kind="ExternalInput")
    W1_d = nc.dram_tensor("W1f", [F_IN, F1], BF16, kind="ExternalInput")
    wsd1_d = nc.dram_tensor("wsd1", [F_IN, 128], BF16, kind="ExternalInput")
    wso1_d = nc.dram_tensor("wso1", [F_IN, 256], BF16, kind="ExternalInput")
    wdn_d = nc.dram_tensor("wdn", [F_IN, 8], BF16, kind="ExternalInput")
    adjU8_d = nc.dram_tensor("adjU8", [N, R], U8, kind="ExternalInput")
    W2a_d = nc.dram_tensor("W2a", [F1, 18], BF16, kind="ExternalInput")
    out_d = nc.dram_tensor("out", [R, D2], FP32, kind="ExternalOutput")

    def dma_chunked(dst_tile, src_ap, inner, batch=False):
        # [C*128, inner] DRAM -> [128, C*inner] SBUF (chunk-major free dim)
        if batch:
            nc.sync.dma_start(
                dst_tile[:].rearrange("p (c o) -> p c o", o=inner),
                src_ap.rearrange("(c p) o -> p c o", p=128),
            )
            return
        nchunks = src_ap.shape[0] // 128
        for cc in range(nchunks):
            nc.sync.dma_start(
                dst_tile[:, cc * inner:(cc + 1) * inner],
                src_ap[cc * 128:(cc + 1) * 128, :],
            )

    with tile.TileContext(nc) as tc:
        with (
            tc.tile_pool(name="const", bufs=1) as cpool,
            tc.tile_pool(name="dram", bufs=1, space="DRAM") as dpool,
        ):
            # ---- resident SBUF tensors (weights first: phase 1 needs them)
            W1_sb = cpool.tile([128, NFC * F1], BF16, tag="W1")
            dma_chunked(W1_sb, W1_d[:], F1)
            wsd1_sb = cpool.tile([128, NFC * 128], BF16, tag="wsd1")
            dma_chunked(wsd1_sb, wsd1_d[:], 128)
            wso1_sb = cpool.tile([128, NFC * 256], BF16, tag="wso1")
            dma_chunked(wso1_sb, wso1_d[:], 256)
            wdn_sb = cpool.tile([128, NFC * 8], BF16, tag="wdn")
            dma_chunked(wdn_sb, wdn_d[:], 8)
            W2a_sb = cpool.tile([128, (F1 // 128) * 18], BF16, tag="W2a")
            dma_chunked(W2a_sb, W2a_d[:], 18)

            adjT_sb = cpool.tile([128, NCH * R], BF16, tag="adjT")

            # on-device constants
            onesb_sb = cpool.tile([128, 128], BF16, tag="onesb")
            nc.vector.memset(onesb_sb[:], 1.0)
            idb_sb = cpool.tile([128, 128], BF16, tag="idb")
            nc.gpsimd.affine_select(
                idb_sb[:], onesb_sb[:], [[-1, 128]],
                mybir.AluOpType.is_equal, 0.0,
                base=0, channel_multiplier=1,
            )
            idBIG_sb = cpool.tile([128, 128], BF16, tag="idBIG")
            nc.vector.tensor_scalar_mul(idBIG_sb[:], idb_sb[:], BIG)
            idf_sb = cpool.tile([16, 16], FP32, tag="idf")
            nc.gpsimd.memset(idf_sb[:], 1.0)
            nc.gpsimd.affine_select(
                idf_sb[:], idf_sb[:], [[-1, 16]],
                mybir.AluOpType.is_equal, 0.0,
                base=0, channel_multiplier=1,
            )
            bigneg = cpool.tile([128, 1], FP32, tag="bigneg")
            nc.vector.memset(bigneg[:], -BIG)

            h_sb = cpool.tile([128, NCH * F1], BF16, tag="h")
            fdT = cpool.tile([128, N], BF16, tag="fdT")
            srcpat = cpool.tile([128, 2 * R], BF16, tag="srcpat")
            vd_sb = cpool.tile([128, NCH * 8], FP32, tag="vd")
            h1T = cpool.tile([128, H1 * R], BF16, tag="h1T")
            h2acc = cpool.tile([128, (R // 128) * 18], FP32, tag="h2acc")
            h2all_sb = cpool.tile([128, NCH * 18], BF16, tag="h2all")
            h2e_sb = cpool.tile([128, NCH * 33], BF16, tag="h2e")
            f2T = cpool.tile([2, N], BF16, tag="f2T")
            srcpat2 = cpool.tile([2, R], BF16, tag="srcpat2")
            fsd2 = cpool.tile([2, R], BF16, tag="fsd2")
            vd2_sb = cpool.tile([128, NCH], FP32, tag="vd2")

            h2loc = dpool.tile([R, 18], BF16, tag="h2loc")
            h2all_d = dpool.tile([N, 18], BF16, tag="h2all",
                                 addr_space="Shared")

            # ones row staged in DRAM (engine writes must be 32-part aligned;
            # DMA writes are not restricted)
            onesN_sb = cpool.tile([1, 1024], BF16, tag="onesN")
            nc.vector.memset(onesN_sb[:], 1.0)
            onesN_d = dpool.tile([1, 1024], BF16, tag="onesNd")
            nc.sync.dma_start(onesN_d[:], onesN_sb[:])

            # tiny warm-up collective: pays CC-core init/skew cost early,
            # overlapped with input DMA + phase 1 (nothing depends on it)
            wup_in = dpool.tile([1, 64], BF16, tag="wupi")
            wup_out = dpool.tile([NCORES, 64], BF16, tag="wupo",
                                 addr_space="Shared")
            nc.sync.dma_start(wup_in[:], onesN_d[:, 0:64])
            nc.gpsimd.collective_compute(
                "AllGather",
                OP.bypass,
                replica_groups=[list(range(NCORES))],
                ins=[wup_in[:].opt()],
                outs=[wup_out[:].opt()],
            )

            # Z-selector: ones in cols 32k..32k+32 put head k's Z at
            # partitions 32k..32k+32 (plain matmuls; no col-tiling)
            zsel_sb = cpool.tile([128, 256], BF16, tag="zsel")
            nc.vector.memset(zsel_sb[:], 0.0)
            nc.vector.memset(zsel_sb[:, 0:32], 1.0)
            nc.vector.memset(zsel_sb[:, 128 + 32:128 + 64], 1.0)

            # srcpat block-diag ones rows (rows 32p+1/+2); zero first
            nc.vector.memset(srcpat[:], 0.0)
            for p in range(4):
                nc.sync.dma_start(
                    srcpat[32 * p + 1:32 * p + 2, 0:R], onesN_d[:, 0:R]
                )
                nc.sync.dma_start(
                    srcpat[32 * p + 2:32 * p + 3, R:2 * R], onesN_d[:, 0:R]
                )

            # =============================================================
            # Phase 1: h = x @ W1 (all nodes); f_dst^T rows via stationary-w
            # matmuls; v_dst = exp(.2 f_dst) columns; own-block .8*f_src^T.
            # x staged in quarters (8 chunks each) to bound SBUF.
            # =============================================================
            with (
                tc.tile_pool(name="xq", bufs=2) as xqpool,
                tc.tile_pool(name="xo", bufs=1) as xopool,
                tc.tile_pool(name="adj8", bufs=1) as adj8pool,
                tc.tile_pool(name="ph", bufs=2, space="PSUM") as php,
                tc.tile_pool(name="pf", bufs=2, space="PSUM") as pfp,
            ):
                # own block .8*f_src^T rows, written at partitions 32p
                xo = xopool.tile([128, 4 * NFC * 128], BF16, tag="xo")
                nc.sync.dma_start(xo[:], xToa_d[:])
                for jt2 in range(R // 128):
                    for k in range(2):
                        pfo = pfp.tile([128, 128], FP32, tag="pft")
                        for fc in range(NFC):
                            nc.tensor.matmul(
                                pfo[:],
                                wso1_sb[:, fc * 256 + 128 * k:
                                        fc * 256 + 128 * (k + 1)],
                                xo[:, (jt2 * NFC + fc) * 128:
                                   (jt2 * NFC + fc + 1) * 128],
                                start=fc == 0, stop=fc == NFC - 1,
                            )
                        for p in range(4):
                            nc.vector.tensor_copy(
                                srcpat[32 * p:32 * p + 1,
                                       k * R + jt2 * 128:
                                       k * R + (jt2 + 1) * 128],
                                pfo[32 * p:32 * p + 1, :],
                            )
                adjU8_sb = adj8pool.tile([128, NCH * R], U8, tag="a8")
                for q in range(4):
                    xq = xqpool.tile([128, 8 * NFC * 128], BF16, tag="xq")
                    nc.sync.dma_start(
                        xq[:], xTa_d[:, q * 8 * NFC * 128:
                                     (q + 1) * 8 * NFC * 128]
                    )
                    # adjacency quarter: DMA + u8->bf16 widening (DVE slack)
                    sl = slice(q * 8 * R, (q + 1) * 8 * R)
                    for cc in range(8):
                        ch = q * 8 + cc
                        nc.sync.dma_start(
                            adjU8_sb[:, ch * R:(ch + 1) * R],
                            adjU8_d[ch * 128:(ch + 1) * 128, :],
                        )
                    nc.vector.tensor_copy(adjT_sb[:, sl], adjU8_sb[:, sl])
                    for jl in range(8):
                        jt = q * 8 + jl
                        ph = php.tile([128, F1], FP32, tag="ph")
                        pft = pfp.tile([128, 128], FP32, tag="pft")
                        pfn = pfp.tile([128, 8], FP32, tag="pfn")
                        for fc in range(NFC):
                            lhs = xq[:, (jl * NFC + fc) * 128:
                                     (jl * NFC + fc + 1) * 128]
                            st = fc == 0
                            sp = fc == NFC - 1
                            nc.tensor.matmul(
                                ph[:, 0:512], lhs,
                                W1_sb[:, fc * F1:fc * F1 + 512],
                                start=st, stop=sp,
                            )
                            nc.tensor.matmul(
                                ph[:, 512:F1], lhs,
                                W1_sb[:, fc * F1 + 512:(fc + 1) * F1],
                                start=st, stop=sp,
                            )
                            nc.tensor.matmul(
                                pfn[:], lhs, wdn_sb[:, fc * 8:(fc + 1) * 8],
                                start=st, stop=sp,
                            )
                            nc.tensor.matmul(
                                pft[:], wsd1_sb[:, fc * 128:(fc + 1) * 128],
                                lhs, start=st, stop=sp,
                            )
                        # evacuate h (split DVE / ACT), f_dst^T, v_dst cols
                        nc.vector.tensor_copy(
                            h_sb[:, jt * F1:jt * F1 + 512], ph[:, 0:512]
                        )
                        nc.scalar.activation(
                            h_sb[:, jt * F1 + 512:(jt + 1) * F1],
                            ph[:, 512:F1], AF.Copy,
                        )
                        nc.vector.tensor_copy(
                            fdT[:, jt * 128:(jt + 1) * 128], pft[:]
                        )
                        nc.scalar.activation(
                            vd_sb[:, jt * 8:(jt + 1) * 8], pfn[:],
                            AF.Exp, scale=ALPHA,
                        )
                # ones rows of fdT (after the copies; WAW-ordered)
                for p in range(4):
                    nc.vector.memset(fdT[32 * p:32 * p + 1, :], 1.0)


            # =============================================================
            # Phase 2: layer-1 attention, 2 heads (one pair) per pass
            # =============================================================
            with (
                tc.tile_pool(name="acc", bufs=1, space="PSUM") as acc,
                tc.tile_pool(name="pe", bufs=2, space="PSUM") as epool,
                tc.tile_pool(name="sc", bufs=6) as spool,
                tc.tile_pool(name="pt", bufs=4) as ptpool,
                tc.tile_pool(name="nrm", bufs=1) as npool,
            ):
                for hp in range(4):
                    p = hp  # pair index; heads (2p, 2p+1)
                    po = [
                        acc.tile([128, R], FP32, tag=f"o{k}", name=f"po{k}")
                        for k in range(2)
                    ]
                    pz = acc.tile([128, R], FP32, tag="z")
                    for c in range(NCH):
                        adj_c = adjT_sb[:, c * R:(c + 1) * R]
                        lhs_e = fdT[32 * p:32 * p + 3, c * 128:(c + 1) * 128]
                        pemask = PEMASK_EVERY and c % PEMASK_EVERY == 0
                        # one [128, 2R] score tile, ONE exp per chunk: the
                        # ScalarE sequencer (~700ns/sem event) is the hidden
                        # serializer — halve its instruction count
                        pec = epool.tile([128, 2 * R], FP32, tag="pe")
                        for k in range(2):
                            nc.tensor.matmul(
                                pec[:, k * R:(k + 1) * R], lhs_e,
                                srcpat[32 * p:32 * p + 3, k * R:(k + 1) * R],
                                start=True, stop=not pemask,
                                tile_position=(32 * p, 0),
                            )
                            if pemask:
                                nc.tensor.matmul(
                                    pec[:, k * R:(k + 1) * R], idBIG_sb[:],
                                    adj_c, start=False, stop=True,
                                )
                        t1 = spool.tile([128, 2 * R], BF16, tag="t1")
                        if pemask:
                            nc.scalar.activation(
                                t1[:], pec[:], AF.Exp, bias=bigneg[:, 0:1]
                            )
                        else:
                            nc.scalar.activation(t1[:], pec[:], AF.Exp)
                        for k in range(2):
                            g = 2 * p + k
                            vd_ap = vd_sb[:, c * 8 + g:c * 8 + g + 1]
                            t1k = t1[:, k * R:(k + 1) * R]
                            pt = ptpool.tile([128, R], BF16, tag="pt")
                            if pemask:
                                nc.vector.scalar_tensor_tensor(
                                    pt[:], adj_c, vd_ap, t1k,
                                    op0=OP.mult, op1=OP.max,
                                )
                            else:
                                q = spool.tile([128, R], BF16, tag="q")
                                nc.vector.tensor_scalar_max(q[:], t1k, vd_ap)
                                eng = (nc.gpsimd
                                       if (2 * c + k) % GPS_EVERY == 0
                                       else nc.vector)
                                eng.tensor_mul(pt[:], q[:], adj_c)
                            nc.tensor.matmul(
                                po[k][:],
                                h_sb[:, c * F1 + g * D1:c * F1 + (g + 1) * D1],
                                pt[:],
                                start=c == 0, stop=c == NCH - 1,
                            )
                            nc.tensor.matmul(
                                pz[:],
                                zsel_sb[:, k * 128:(k + 1) * 128],
                                pt[:],
                                start=(c == 0 and k == 0),
                                stop=(c == NCH - 1 and k == 1),
                            )
                    # normalize + ELU -> h1^T (bf16).  Evacuate po/pz to
                    # SBUF immediately so the accumulator banks free up for
                    # the next pair (PE keeps streaming, stays HAM-warm).
                    zinv = npool.tile([64, R], FP32, tag="zinv")
                    nc.vector.reciprocal_approx_fast(zinv[:], pz[0:64, :])
                    zinvb = npool.tile([64, R], BF16, tag="zinvb")
                    nc.vector.tensor_copy(zinvb[:], zinv[:])
                    po_sb = [npool.tile([128, R], FP32, tag=f"posb{k}",
                                        name=f"po_sb{k}")
                             for k in range(2)]
                    nc.vector.tensor_copy(po_sb[0][:], po[0][:])
                    nc.scalar.activation(po_sb[1][:], po[1][:], AF.Copy)
                    for k in range(2):
                        g = 2 * p + k
                        zb_psf = epool.tile([128, 2 * R], FP32, tag="pe")
                        zb_ps = zb_psf[:, 0:R]
                        nc.tensor.matmul(
                            zb_ps,
                            onesb_sb[32 * k:32 * k + 1, :],
                            zinvb[32 * k:32 * k + 1, :],
                            start=True, stop=True,
                            tile_position=(32 * k, 0),
                        )
                        zb_sb = npool.tile([128, R], FP32, tag="zbs")
                        nc.vector.tensor_copy(zb_sb[:], zb_ps)
                        pre = npool.tile([128, R], FP32, tag="pre")
                        nc.vector.tensor_mul(pre[:], po_sb[k][:], zb_sb[:])
                        r = npool.tile([128, R], FP32, tag="r")
                        nc.scalar.activation(r[:], pre[:], AF.Relu, scale=-1.0)
                        t = npool.tile([128, R], FP32, tag="t")
                        nc.scalar.activation(t[:], r[:], AF.Exp, scale=-1.0)
                        u = npool.tile([128, R], FP32, tag="u")
                        nc.vector.tensor_scalar_add(u[:], t[:], -1.0)
                        nc.vector.tensor_max(
                            h1T[:, g * R:(g + 1) * R], pre[:], u[:]
                        )
                    # h2 partial: accumulate this pair's 2 feature blocks
                    for jt2 in range(R // 128):
                        ph2f = epool.tile([128, 2 * R], FP32, tag="pe")
                        ph2 = ph2f[:, 0:18]
                        for k in range(2):
                            g = 2 * p + k
                            nc.tensor.matmul(
                                ph2,
                                h1T[:, g * R + jt2 * 128:
                                    g * R + (jt2 + 1) * 128],
                                W2a_sb[:, g * 18:(g + 1) * 18],
                                start=k == 0, stop=k == 1,
                            )
                        dst = h2acc[:, jt2 * 18:(jt2 + 1) * 18]
                        if p == 0:
                            nc.vector.tensor_copy(dst, ph2)
                        else:
                            nc.vector.tensor_add(dst, dst, ph2)

            # =============================================================
            # Phase 3: AllGather h2 ([N, 18]); rebuild f2 rows
            # =============================================================
            with (
                tc.tile_pool(name="p2t", bufs=2, space="PSUM") as p2tp,
                tc.tile_pool(name="h2s", bufs=2) as h2p,
            ):
                for jt2 in range(R // 128):
                    h2t = h2p.tile([128, 18], BF16, tag="h2t")
                    nc.vector.tensor_copy(
                        h2t[:], h2acc[:, jt2 * 18:(jt2 + 1) * 18]
                    )
                    nc.sync.dma_start(
                        h2loc[jt2 * 128:(jt2 + 1) * 128, :], h2t[:]
                    )
                    # transpose cols 16:18 -> [fs2 row; fd2 row] (local)
                    ps2 = p2tp.tile([2, 128], BF16, tag="ps2")
                    nc.tensor.transpose(ps2[:], h2t[:, 16:18], idb_sb[:])
                    nc.vector.tensor_copy(
                        fsd2[0:2, jt2 * 128:(jt2 + 1) * 128], ps2[0:2, :]
                    )
                nc.gpsimd.collective_compute(
                    "AllGather",
                    OP.bypass,
                    replica_groups=[list(range(NCORES))],
                    ins=[h2loc[:].opt()],
                    outs=[h2all_d[:].opt()],
                )
                dma_chunked(h2all_sb, h2all_d[:], 18, batch=True)
                for q4 in range(4):
                    nc.sync.dma_start(
                        f2T[1:2, q4 * 1024:(q4 + 1) * 1024], onesN_d[:]
                    )
                # srcpat2 rows: [ones; .8*fs2 (local, pre-scaled via W2a)]
                nc.vector.memset(srcpat2[0:1, :], 1.0)
                nc.sync.dma_start(srcpat2[1:2, :], fsd2[0:1, :])
                # layer-2 exp(.2 f_dst2) columns
                nc.scalar.activation(
                    vd2_sb[:, 0:NCH], h2all_sb[:, 17:NCH * 18:18],
                    AF.Exp, scale=ALPHA,
                )
                # h2e = [h2 (16) | zeros (16) | ones] per chunk: the ones
                # column puts Z2 at PSUM partition 32 (32-aligned for reads)
                nc.vector.memset(h2e_sb[:], 0.0)
                nc.vector.tensor_copy(
                    h2e_sb[:].rearrange("p (c o) -> p c o", o=33)[:, :, 0:16],
                    h2all_sb[:].rearrange("p (c o) -> p c o", o=18)[:, :, 0:16],
                )
                nc.vector.memset(
                    h2e_sb[:].rearrange("p (c o) -> p c o", o=33)[:, :, 32:33],
                    1.0,
                )

            # =============================================================
            # Phase 4: layer-2 attention + ELU + log_softmax
            # =============================================================
            with (
                tc.tile_pool(name="acc2", bufs=1, space="PSUM") as acc2,
                tc.tile_pool(name="pe2", bufs=2, space="PSUM") as e2pool,
                tc.tile_pool(name="sc2", bufs=3) as spool2,
                tc.tile_pool(name="fin", bufs=2) as fpool,
                tc.tile_pool(name="pfin", bufs=1, space="PSUM") as pfp2,
                tc.tile_pool(name="pc", bufs=2, space="PSUM") as pcp,
            ):
                po2 = acc2.tile([64, R], FP32, tag="o2")
                for c in range(NCH):
                    adj_c = adjT_sb[:, c * R:(c + 1) * R]
                    # f2T fd2-row chunk from h2all col 17 (interleaved so
                    # chunk 0's scores don't wait on all 32 transposes)
                    pcol = pcp.tile([1, 128], BF16, tag="pcol")
                    nc.tensor.transpose(
                        pcol[:], h2all_sb[:, c * 18 + 17:c * 18 + 18],
                        idb_sb[:],
                    )
                    nc.vector.tensor_copy(
                        f2T[0:1, c * 128:(c + 1) * 128], pcol[:]
                    )
                    pe2 = e2pool.tile([128, R], FP32, tag="pe2")
                    nc.tensor.matmul(
                        pe2[:], f2T[:, c * 128:(c + 1) * 128], srcpat2[:],
                        start=True, stop=False,
                    )
                    nc.tensor.matmul(
                        pe2[:], idBIG_sb[:], adj_c,
                        start=False, stop=True,
                    )
                    t1 = spool2.tile([128, R], BF16, tag="t1b")
                    nc.scalar.activation(
                        t1[:], pe2[:], AF.Exp, bias=bigneg[:, 0:1]
                    )
                    pt2 = spool2.tile([128, R], BF16, tag="pt2")
                    nc.vector.scalar_tensor_tensor(
                        pt2[:], adj_c, vd2_sb[:, c:c + 1], t1[:],
                        op0=OP.mult, op1=OP.max,
                    )
                    nc.tensor.matmul(
                        po2[0:33, :], h2e_sb[:, c * 33:(c + 1) * 33], pt2[:],
                        start=c == 0, stop=c == NCH - 1,
                    )
                # approx-reciprocal misreads PSUM at a non-zero partition
                # base; stage the Z2 row to SBUF partition 0 first
                z2sb = fpool.tile([1, R], FP32, tag="z2sb")
                nc.vector.tensor_copy(z2sb[:], po2[32:33, :])
                zinv2 = fpool.tile([1, R], FP32, tag="zinv2")
                nc.vector.reciprocal_approx_fast(zinv2[:], z2sb[:])
                zinv2b = fpool.tile([1, R], BF16, tag="zinv2b")
                nc.vector.tensor_copy(zinv2b[:], zinv2[:])
                zb2_ps = pfp2.tile([16, R], FP32, tag="zb2")
                nc.tensor.matmul(
                    zb2_ps[:], onesb_sb[0:1, 0:16], zinv2b[:],
                    start=True, stop=True,
                )
                zb2 = fpool.tile([16, R], FP32, tag="zb2s")
                nc.vector.tensor_copy(zb2[:], zb2_ps[:])
                pre2 = fpool.tile([16, R], FP32, tag="pre2")
                nc.vector.tensor_mul(pre2[:], po2[0:16, :], zb2[:])
                r2 = fpool.tile([16, R], FP32, tag="r2")
                nc.scalar.activation(r2[:], pre2[:], AF.Relu, scale=-1.0)
                t2e = fpool.tile([16, R], FP32, tag="t2e")
                nc.scalar.activation(t2e[:], r2[:], AF.Exp, scale=-1.0)
                u2 = fpool.tile([16, R], FP32, tag="u2")
                nc.vector.tensor_scalar_add(u2[:], t2e[:], -1.0)
                elu2 = fpool.tile([16, R], FP32, tag="elu2")
                nc.vector.tensor_max(elu2[:], pre2[:], u2[:])
                # transpose to natural [i, o2] then log_softmax over free
                # dim; exps batched before one Ln (fewer ACT table switches)
                pns, nmxs = [], []
                s_all = fpool.tile([128, 4], FP32, tag="s_all")
                for it in range(R // 128):
                    pn = fpool.tile([128, 16], FP32, tag=f"pn{it}",
                                    name=f"pn{it}")
                    pnp = pfp2.tile([128, 16], FP32, tag="pn")
                    nc.tensor.transpose(
                        pnp[:], elu2[:, it * 128:(it + 1) * 128],
                        idf_sb[:],
                    )
                    nc.vector.tensor_copy(pn[:], pnp[:])
                    nmx = fpool.tile([128, 1], FP32, tag=f"nmx{it}",
                                     name=f"nmx{it}")
                    nc.vector.tensor_reduce(
                        nmx[:], pn[:], AX.X, OP.max, negate=True
                    )
                    ex = fpool.tile([128, 16], FP32, tag="ex")
                    nc.scalar.activation(
                        ex[:], pn[:], AF.Exp, bias=nmx[:, 0:1],
                        accum_out=s_all[:, it:it + 1],
                    )
                    pns.append(pn); nmxs.append(nmx)
                lg = fpool.tile([128, 4], FP32, tag="lg")
                nc.scalar.activation(lg[:], s_all[:], AF.Ln)
                for it in range(R // 128):
                    fin = fpool.tile([128, 16], FP32, tag="fin")
                    nc.vector.tensor_scalar(
                        fin[:], pns[it][:], nmxs[it][:, 0:1], lg[:, it:it + 1],
                        op0=OP.add, op1=OP.subtract,
                    )
                    nc.sync.dma_start(out_d[it * 128:(it + 1) * 128, :], fin[:])

    nc.compile()
    return nc


def _get_nc():
    if "nc" not in _BUILD_CACHE:
        _BUILD_CACHE["nc"] = _build_nc()
    return _BUILD_CACHE["nc"]


def _prep_inputs(x, adj, W1, a_src1, a_dst1, W2, a_src2, a_dst2):
    bf16 = ml_dtypes.bfloat16
    f32 = np.float32
    x = np.asarray(x, f32)
    adj = np.asarray(adj, f32)
    W1 = np.asarray(W1, f32)
    W2 = np.asarray(W2, f32)
    a_src1 = np.asarray(a_src1, f32)
    a_dst1 = np.asarray(a_dst1, f32)
    a_src2 = np.asarray(a_src2, f32)
    a_dst2 = np.asarray(a_dst2, f32)

    W1f = np.ascontiguousarray(W1.reshape(F_IN, F1))
    # folded score vectors: f_src[h] = x @ (W1[:,h,:] @ a_src1[h])
    wsrc = np.stack([W1[:, h, :] @ a_src1[h] for h in range(H1)], axis=1)
    wdst = np.stack([W1[:, h, :] @ a_dst1[h] for h in range(H1)], axis=1)
    # pair p lives at partitions 32p..32p+2: [ones, fd_2p, fd_2p+1]
    wsd1 = np.zeros((F_IN, 128), f32)
    for p in range(4):
        wsd1[:, 32 * p + 1] = wdst[:, 2 * p]
        wsd1[:, 32 * p + 2] = wdst[:, 2 * p + 1]
    # .8*f_src columns placed so the transform emits rows at partition 32p
    wso1 = np.zeros((F_IN, 256), f32)
    for k in range(2):
        for p in range(4):
            wso1[:, 128 * k + 32 * p] = 0.8 * wsrc[:, 2 * p + k]
    W2f = np.ascontiguousarray(W2.reshape(F1, D2))
    W2a = np.zeros((F1, 18), f32)
    W2a[:, :D2] = W2f
    W2a[:, 16] = 0.8 * (W2f @ a_src2[0])
    W2a[:, 17] = W2f @ a_dst2[0]

    adjT_u8 = (adj.T > 0).astype(np.uint8)
    xb = x.astype(bf16)
    # xTa[p, (jt, fc, o)] = x[jt*128+o, fc*128+p]
    xTa = np.ascontiguousarray(
        xb.reshape(NCH, 128, NFC, 128).transpose(3, 0, 2, 1)
    ).reshape(128, NCH * NFC * 128)

    shared = {
        "W1f": W1f.astype(bf16),
        "wsd1": wsd1.astype(bf16),
        "wso1": wso1.astype(bf16),
        "wdn": wdst.astype(bf16),
        "W2a": W2a.astype(bf16),
        "xTa": xTa,
    }
    in_maps = []
    for c in range(NCORES):
        blkslice = slice(c * R, (c + 1) * R)
        m = dict(shared)
        m["adjU8"] = np.ascontiguousarray(adjT_u8[:, blkslice])
        m["xToa"] = np.ascontiguousarray(
            xTa[:, c * 4 * NFC * 128:(c + 1) * 4 * NFC * 128]
        )
        in_maps.append(m)
    return in_maps


def kernel(x, adj, W1, a_src1, a_dst1, W2, a_src2, a_dst2, _trace=False):
    from concourse.bass_utils import run_bass_kernel_spmd

    nc = _get_nc()
    in_maps = _prep_inputs(x, adj, W1, a_src1, a_dst1, W2, a_src2, a_dst2)
    res = run_bass_kernel_spmd(nc, in_maps, list(range(NCORES)), trace=_trace)
    out = np.concatenate(
        [np.asarray(res.results[c]["out"]) for c in range(NCORES)], axis=0
    )
    kernel.last_results = res
    return out.astype(np.float32)


# revision 33
# speedup vs baseline: 1.1987x; 1.0622x over previous
"""Two-layer GAT (8-head + 1-head) Trainium2 Bass kernel, 8-way node-sharded.

Strategy (per core c, owning row block I_c of R = N/8 nodes):
  * Scores live in TRANSPOSED layout [j (partition), i (free)] so the
    aggregation matmul out^T[o, i] = sum_j h[j, o] * P[j, i] contracts over
    the partition dim naturally (lhsT = h rows, rhs = P^T tile).
  * Softmax factorization: exp(lrelu(fs[i] + fd[j])) =
      exp(.2 fs[i]) * max(exp(.8 fs[i] + fd[j]), exp(.2 fd[j]))
    and the exp(.2 fs[i]) factor is constant along the softmax axis j, so it
    cancels in P/Z.
  * The adjacency mask folds into the scores on the TensorEngine: the score
    PSUM accumulates BIG*adj (lhsT = BIG*I, rhs = adjT chunk) and the exp
    bias subtracts BIG, so exp(u + BIG*(adj-1)) ~ 0 off-edges.  Per score
    tile the elementwise work is then ONE ScalarE exp plus ONE fused DVE op:
      P = (adjT * vd[j]) max exp(u'')     (scalar_tensor_tensor)
    with a GpSimd share.  This also keeps the PE stream dense (HAM-warm).
  * u[j, i] = .8 fs[i] + fd[j] comes from a rank-3 matmul (lhsT rows =
    [ones, fd_h0, fd_h1] at a 32-aligned partition base, rhs = packed
    .8*fs row + block-diag ones rows), two heads per PSUM tile.
  * Z_i = sum_j P[j, i] via col-tiled ones matmuls (two heads in disjoint
    32-partition blocks of one PSUM tile, issued adjacent for overlap).
  * f vectors fall out of the feature transform for free by running it with
    the folded vector w = W @ a as the STATIONARY operand and the x^T tile
    moving: the result lands pre-transposed at exactly the partitions the
    score matmuls need.
  * x is uploaded pre-arranged to the exact SBUF layout (one contiguous DMA
    slice per chunk); adjacency is uploaded uint8 and widened on-device.
  * Layer-2 h2 = h1 @ W2 partials accumulate after each head-pair pass; one
    AllGather distributes h2 ([N, 18]).  Layer-2 aggregation and Z2 come
    from ONE matmul with stationary [h2 | 0 | ones] (Z2 at partition 32).
"""

import sys

sys.path.insert(0, "/opt/trn_rl_repo")

import numpy as np
import ml_dtypes

N = 4096
F_IN = 512
H1 = 8
D1 = 128
F1 = 1024          # H1 * D1
D2 = 16
NCORES = 8
R = N // NCORES    # rows (nodes) per core
NCH = N // 128     # j-chunks of 128
NFC = F_IN // 128  # f chunks
ALPHA = 0.2
BIG = 30.0         # mask shift: exp(-BIG + u) ~ 0 for off-edges

GPS_EVERY = 4      # gpsimd takes the mask multiply on every 4th sub-step
PEMASK_EVERY = 4   # chunks where the PE accumulates BIG*adj (fused DVE op)

_BUILD_CACHE = {}


def _build_nc():
    import concourse.bacc as bacc
    import concourse.tile as tile
    import concourse.mybir as mybir

    FP32 = mybir.dt.float32
    BF16 = mybir.dt.bfloat16
    U8 = mybir.dt.uint8
    AF = mybir.ActivationFunctionType
    OP = mybir.AluOpType
    AX = mybir.AxisListType

    nc = bacc.Bacc(num_devices=NCORES)

    # ---- I/O -------------------------------------------------------------
    # xTa[p, (jt, fc, o)] = x[jt*128+o, fc*128+p]  (pre-arranged SBUF layout)
    xTa_d = nc.dram_tensor("xTa", [128, NCH * NFC * 128], BF16,
                           kind="ExternalInput")
    # xToa: same layout for the core's own 4 chunks
    xToa_d = nc.dram_tensor("xToa", [128, 4 * NFC * 128], BF16,
                            kind="ExternalInput")
    W1_d = nc.dram_tensor("W1f", [F_IN, F1], BF16, kind="ExternalInput")
    wsd1_d = nc.dram_tensor("wsd1", [F_IN, 128], BF16, kind="ExternalInput")
    wso1_d = nc.dram_tensor("wso1", [F_IN, 256], BF16, kind="ExternalInput")
    wdn_d = nc.dram_tensor("wdn", [F_IN, 8], BF16, kind="ExternalInput")
    adjU8_d = nc.dram_tensor("adjU8", [N, R], U8, kind="ExternalInput")
    W2a_d = nc.dram_tensor("W2a", [F1, 18], BF16, kind="ExternalInput")
    out_d = nc.dram_tensor("out", [R, D2], FP32, kind="ExternalOutput")

    def dma_chunked(dst_tile, src_ap, inner, batch=False):
        # [C*128, inner] DRAM -> [128, C*inner] SBUF (chunk-major free dim)
        if batch:
            nc.sync.dma_start(
                dst_tile[:].rearrange("p (c o) -> p c o", o=inner),
                src_ap.rearrange("(c p) o -> p c o", p=128),
            )
            return
        nchunks = src_ap.shape[0] // 128
        for cc in range(nchunks):
            nc.sync.dma_start(
                dst_tile[:, cc * inner:(cc + 1) * inner],
                src_ap[cc * 128:(cc + 1) * 128, :],
            )

    with tile.TileContext(nc) as tc:
        with (
            tc.tile_pool(name="const", bufs=1) as cpool,
            tc.tile_pool(name="dram", bufs=1, space="DRAM") as dpool,
        ):
            # ---- resident SBUF tensors (weights first: phase 1 needs them)
            W1_sb = cpool.tile([128, NFC * F1], BF16, tag="W1")
            dma_chunked(W1_sb, W1_d[:], F1)
            wsd1_sb = cpool.tile([128, NFC * 128], BF16, tag="wsd1")
            dma_chunked(wsd1_sb, wsd1_d[:], 128)
            wso1_sb = cpool.tile([128, NFC * 256], BF16, tag="wso1")
            dma_chunked(wso1_sb, wso1_d[:], 256)
            wdn_sb = cpool.tile([128, NFC * 8], BF16, tag="wdn")
            dma_chunked(wdn_sb, wdn_d[:], 8)
            W2a_sb = cpool.tile([128, (F1 // 128) * 18], BF16, tag="W2a")
            dma_chunked(W2a_sb, W2a_d[:], 18)

            adjT_sb = cpool.tile([128, NCH * R], BF16, tag="adjT")

            # on-device constants
            onesb_sb = cpool.tile([128, 128], BF16, tag="onesb")
            nc.vector.memset(onesb_sb[:], 1.0)
            idb_sb = cpool.tile([128, 128], BF16, tag="idb")
            nc.gpsimd.affine_select(
                idb_sb[:], onesb_sb[:], [[-1, 128]],
                mybir.AluOpType.is_equal, 0.0,
                base=0, channel_multiplier=1,
            )
            idBIG_sb = cpool.tile([128, 128], BF16, tag="idBIG")
            nc.vector.tensor_scalar_mul(idBIG_sb[:], idb_sb[:], BIG)
            idf_sb = cpool.tile([16, 16], FP32, tag="idf")
            nc.gpsimd.memset(idf_sb[:], 1.0)
            nc.gpsimd.affine_select(
                idf_sb[:], idf_sb[:], [[-1, 16]],
                mybir.AluOpType.is_equal, 0.0,
                base=0, channel_multiplier=1,
            )
            bigneg = cpool.tile([128, 1], FP32, tag="bigneg")
            nc.vector.memset(bigneg[:], -BIG)

            h_sb = cpool.tile([128, NCH * F1], BF16, tag="h")
            fdT = cpool.tile([128, N], BF16, tag="fdT")
            srcpat = cpool.tile([128, 2 * R], BF16, tag="srcpat")
            vd_sb = cpool.tile([128, NCH * 8], FP32, tag="vd")
            h1T = cpool.tile([128, H1 * R], BF16, tag="h1T")
            h2acc = cpool.tile([128, (R // 128) * 18], FP32, tag="h2acc")
            h2all_sb = cpool.tile([128, NCH * 18], BF16, tag="h2all")
            h2e_sb = cpool.tile([128, NCH * 33], BF16, tag="h2e")
            f2T = cpool.tile([2, N], BF16, tag="f2T")
            srcpat2 = cpool.tile([2, R], BF16, tag="srcpat2")
            fsd2 = cpool.tile([2, R], BF16, tag="fsd2")
            vd2_sb = cpool.tile([128, NCH], FP32, tag="vd2")

            h2loc = dpool.tile([R, 18], BF16, tag="h2loc")
            h2all_d = dpool.tile([N, 18], BF16, tag="h2all",
                                 addr_space="Shared")

            # ones row staged in DRAM (engine writes must be 32-part aligned;
            # DMA writes are not restricted)
            onesN_sb = cpool.tile([1, 1024], BF16, tag="onesN")
            nc.vector.memset(onesN_sb[:], 1.0)
            onesN_d = dpool.tile([1, 1024], BF16, tag="onesNd")
            nc.sync.dma_start(onesN_d[:], onesN_sb[:])

            # tiny warm-up collective: pays CC-core init/skew cost early,
            # overlapped with input DMA + phase 1 (nothing depends on it)
            wup_in = dpool.tile([1, 64], BF16, tag="wupi")
            wup_out = dpool.tile([NCORES, 64], BF16, tag="wupo",
                                 addr_space="Shared")
            nc.sync.dma_start(wup_in[:], onesN_d[:, 0:64])
            nc.gpsimd.collective_compute(
                "AllGather",
                OP.bypass,
                replica_groups=[list(range(NCORES))],
                ins=[wup_in[:].opt()],
                outs=[wup_out[:].opt()],
            )

            # Z-selector: ones in cols 32k..32k+32 put head k's Z at
            # partitions 32k..32k+32 (plain matmuls; no col-tiling)
            zsel_sb = cpool.tile([128, 256], BF16, tag="zsel")
            nc.vector.memset(zsel_sb[:], 0.0)
            nc.vector.memset(zsel_sb[:, 0:32], 1.0)
            nc.vector.memset(zsel_sb[:, 128 + 32:128 + 64], 1.0)

            # srcpat block-diag ones rows (rows 32p+1/+2); zero first
            nc.vector.memset(srcpat[:], 0.0)
            for p in range(4):
                nc.sync.dma_start(
                    srcpat[32 * p + 1:32 * p + 2, 0:R], onesN_d[:, 0:R]
                )
                nc.sync.dma_start(
                    srcpat[32 * p + 2:32 * p + 3, R:2 * R], onesN_d[:, 0:R]
                )

            # =============================================================
            # Phase 1: h = x @ W1 (all nodes); f_dst^T rows via stationary-w
            # matmuls; v_dst = exp(.2 f_dst) columns; own-block .8*f_src^T.
            # x staged in quarters (8 chunks each) to bound SBUF.
            # =============================================================
            with (
                tc.tile_pool(name="xq", bufs=2) as xqpool,
                tc.tile_pool(name="xo", bufs=1) as xopool,
                tc.tile_pool(name="adj8", bufs=1) as adj8pool,
                tc.tile_pool(name="ph", bufs=2, space="PSUM") as php,
                tc.tile_pool(name="pf", bufs=2, space="PSUM") as pfp,
            ):
                # own block .8*f_src^T rows, written at partitions 32p
                xo = xopool.tile([128, 4 * NFC * 128], BF16, tag="xo")
                nc.sync.dma_start(xo[:], xToa_d[:])
                for jt2 in range(R // 128):
                    for k in range(2):
                        pfo = pfp.tile([128, 128], FP32, tag="pft")
                        for fc in range(NFC):
                            nc.tensor.matmul(
                                pfo[:],
                                wso1_sb[:, fc * 256 + 128 * k:
                                        fc * 256 + 128 * (k + 1)],
                                xo[:, (jt2 * NFC + fc) * 128:
                                   (jt2 * NFC + fc + 1) * 128],
                                start=fc == 0, stop=fc == NFC - 1,
                            )
                        for p in range(4):
                            nc.vector.tensor_copy(
                                srcpat[32 * p:32 * p + 1,
                                       k * R + jt2 * 128:
                                       k * R + (jt2 + 1) * 128],
                                pfo[32 * p:32 * p + 1, :],
                            )
                adjU8_sb = adj8pool.tile([128, NCH * R], U8, tag="a8")
                for q in range(4):
                    xq = xqpool.tile([128, 8 * NFC * 128], BF16, tag="xq")
                    nc.sync.dma_start(
                        xq[:], xTa_d[:, q * 8 * NFC * 128:
                                     (q + 1) * 8 * NFC * 128]
                    )
                    # adjacency quarter: DMA + u8->bf16 widening (DVE slack)
                    sl = slice(q * 8 * R, (q + 1) * 8 * R)
                    for cc in range(8):
                        ch = q * 8 + cc
                        nc.sync.dma_start(
                            adjU8_sb[:, ch * R:(ch + 1) * R],
                            adjU8_d[ch * 128:(ch + 1) * 128, :],
                        )
                    nc.vector.tensor_copy(adjT_sb[:, sl], adjU8_sb[:, sl])
                    for jl in range(8):
                        jt = q * 8 + jl
                        ph = php.tile([128, F1], FP32, tag="ph")
                        pft = pfp.tile([128, 128], FP32, tag="pft")
                        pfn = pfp.tile([128, 8], FP32, tag="pfn")
                        for fc in range(NFC):
                            lhs = xq[:, (jl * NFC + fc) * 128:
                                     (jl * NFC + fc + 1) * 128]
                            st = fc == 0
                            sp = fc == NFC - 1
                            nc.tensor.matmul(
                                ph[:, 0:512], lhs,
                                W1_sb[:, fc * F1:fc * F1 + 512],
                                start=st, stop=sp,
                            )
                            nc.tensor.matmul(
                                ph[:, 512:F1], lhs,
                                W1_sb[:, fc * F1 + 512:(fc + 1) * F1],
                                start=st, stop=sp,
                            )
                            nc.tensor.matmul(
                                pfn[:], lhs, wdn_sb[:, fc * 8:(fc + 1) * 8],
                                start=st, stop=sp,
                            )
                            nc.tensor.matmul(
                                pft[:], wsd1_sb[:, fc * 128:(fc + 1) * 128],
                                lhs, start=st, stop=sp,
                            )
                        # evacuate h (split DVE / ACT), f_dst^T, v_dst cols
                        nc.vector.tensor_copy(
                            h_sb[:, jt * F1:jt * F1 + 512], ph[:, 0:512]
                        )
                        nc.scalar.activation(
                            h_sb[:, jt * F1 + 512:(jt + 1) * F1],
                            ph[:, 512:F1], AF.Copy,
                        )
                        nc.vector.tensor_copy(
                            fdT[:, jt * 128:(jt + 1) * 128], pft[:]
                        )
                        nc.scalar.activation(
                            vd_sb[:, jt * 8:(jt + 1) * 8], pfn[:],
                            AF.Exp, scale=ALPHA,
                        )
                # ones rows of fdT (after the copies; WAW-ordered)
                for p in range(4):
                    nc.vector.memset(fdT[32 * p:32 * p + 1, :], 1.0)


            # =============================================================
            # Phase 2: layer-1 attention, 2 heads (one pair) per pass
            # =============================================================
            with (
                tc.tile_pool(name="acc", bufs=1, space="PSUM") as acc,
                tc.tile_pool(name="pe", bufs=2, space="PSUM") as epool,
                tc.tile_pool(name="sc", bufs=8) as spool,
                tc.tile_pool(name="pt", bufs=6) as ptpool,
                tc.tile_pool(name="nrm", bufs=1) as npool,
            ):
                for half in range(2):  # two pairs interleaved per pass
                    po = {}
                    pzs = {}
                    for pr in range(2):
                        po[pr] = [
                            acc.tile([128, R], FP32, tag=f"o{pr}{k}",
                                     name=f"po{pr}{k}")
                            for k in range(2)
                        ]
                        pzs[pr] = acc.tile([128, R], FP32, tag=f"z{pr}",
                                           name=f"pz{pr}")
                    for c in range(NCH):
                        adj_c = adjT_sb[:, c * R:(c + 1) * R]
                        pemask = PEMASK_EVERY and c % PEMASK_EVERY == 0
                        for pr in range(2):
                            p = 2 * half + pr
                            lhs_e = fdT[32 * p:32 * p + 3,
                                        c * 128:(c + 1) * 128]
                            for k in range(2):
                                g = 2 * p + k
                                vd_ap = vd_sb[:, c * 8 + g:c * 8 + g + 1]
                                pek = epool.tile([128, R], FP32, tag="pe")
                                nc.tensor.matmul(
                                    pek[:], lhs_e,
                                    srcpat[32 * p:32 * p + 3,
                                           k * R:(k + 1) * R],
                                    start=True, stop=not pemask,
                                    tile_position=(32 * p, 0),
                                )
                                if pemask:
                                    nc.tensor.matmul(
                                        pek[:], idBIG_sb[:], adj_c,
                                        start=False, stop=True,
                                    )
                                t1 = spool.tile([128, R], BF16, tag="t1")
                                if pemask:
                                    nc.scalar.activation(
                                        t1[:], pek[:], AF.Exp,
                                        bias=bigneg[:, 0:1],
                                    )
                                else:
                                    nc.scalar.activation(t1[:], pek[:], AF.Exp)
                                pt = ptpool.tile([128, R], BF16, tag="pt")
                                if pemask:
                                    nc.vector.scalar_tensor_tensor(
                                        pt[:], adj_c, vd_ap, t1[:],
                                        op0=OP.mult, op1=OP.max,
                                    )
                                else:
                                    q = spool.tile([128, R], BF16, tag="q")
                                    nc.vector.tensor_scalar_max(
                                        q[:], t1[:], vd_ap
                                    )
                                    eng = (nc.gpsimd
                                           if (4 * c + 2 * pr + k)
                                           % GPS_EVERY == 0
                                           else nc.vector)
                                    eng.tensor_mul(pt[:], q[:], adj_c)
                                nc.tensor.matmul(
                                    po[pr][k][:],
                                    h_sb[:, c * F1 + g * D1:
                                         c * F1 + (g + 1) * D1],
                                    pt[:],
                                    start=c == 0, stop=c == NCH - 1,
                                )
                                nc.tensor.matmul(
                                    pzs[pr][:],
                                    zsel_sb[:, k * 128:(k + 1) * 128],
                                    pt[:],
                                    start=(c == 0 and k == 0),
                                    stop=(c == NCH - 1 and k == 1),
                                )
                    # normalize + ELU -> h1^T (bf16), both pairs
                    for pr in range(2):
                        p = 2 * half + pr
                        pz = pzs[pr]
                        zinv = npool.tile([64, R], FP32, tag="zinv")
                        nc.vector.reciprocal_approx_fast(zinv[:], pz[0:64, :])
                        zinvb = npool.tile([64, R], BF16, tag="zinvb")
                        nc.vector.tensor_copy(zinvb[:], zinv[:])
                        po_sb = [npool.tile([128, R], FP32, tag=f"posb{k}",
                                            name=f"po_sb{k}")
                                 for k in range(2)]
                        nc.vector.tensor_copy(po_sb[0][:], po[pr][0][:])
                        nc.scalar.activation(po_sb[1][:], po[pr][1][:],
                                             AF.Copy)
                        for k in range(2):
                            g = 2 * p + k
                            zb_ps = epool.tile([128, R], FP32, tag="pe")
                            nc.tensor.matmul(
                                zb_ps[:],
                                onesb_sb[32 * k:32 * k + 1, :],
                                zinvb[32 * k:32 * k + 1, :],
                                start=True, stop=True,
                                tile_position=(32 * k, 0),
                            )
                            zb_sb = npool.tile([128, R], FP32, tag="zbs")
                            nc.vector.tensor_copy(zb_sb[:], zb_ps[:])
                            pre = npool.tile([128, R], FP32, tag="pre")
                            nc.vector.tensor_mul(pre[:], po_sb[k][:],
                                                 zb_sb[:])
                            r = npool.tile([128, R], FP32, tag="r")
                            nc.scalar.activation(r[:], pre[:], AF.Relu,
                                                 scale=-1.0)
                            t = npool.tile([128, R], FP32, tag="t")
                            nc.scalar.activation(t[:], r[:], AF.Exp,
                                                 scale=-1.0)
                            u = npool.tile([128, R], FP32, tag="u")
                            nc.vector.tensor_scalar_add(u[:], t[:], -1.0)
                            nc.vector.tensor_max(
                                h1T[:, g * R:(g + 1) * R], pre[:], u[:]
                            )
                    # h2 partial: this half's 4 feature blocks
                    for jt2 in range(R // 128):
                        ph2f = epool.tile([128, R], FP32, tag="pe")
                        ph2 = ph2f[:, 0:18]
                        for gi in range(4):
                            g = 4 * half + gi
                            nc.tensor.matmul(
                                ph2,
                                h1T[:, g * R + jt2 * 128:
                                    g * R + (jt2 + 1) * 128],
                                W2a_sb[:, g * 18:(g + 1) * 18],
                                start=gi == 0, stop=gi == 3,
                            )
                        dst = h2acc[:, jt2 * 18:(jt2 + 1) * 18]
                        if half == 0:
                            nc.vector.tensor_copy(dst, ph2)
                        else:
                            nc.vector.tensor_add(dst, dst, ph2)

            # =============================================================
            # Phase 3: AllGather h2 ([N, 18]); rebuild f2 rows
            # =============================================================
            with (
                tc.tile_pool(name="p2t", bufs=2, space="PSUM") as p2tp,
                tc.tile_pool(name="h2s", bufs=2) as h2p,
            ):
                for jt2 in range(R // 128):
                    h2t = h2p.tile([128, 18], BF16, tag="h2t")
                    nc.vector.tensor_copy(
                        h2t[:], h2acc[:, jt2 * 18:(jt2 + 1) * 18]
                    )
                    nc.sync.dma_start(
                        h2loc[jt2 * 128:(jt2 + 1) * 128, :], h2t[:]
                    )
                    # transpose cols 16:18 -> [fs2 row; fd2 row] (local)
                    ps2 = p2tp.tile([2, 128], BF16, tag="ps2")
                    nc.tensor.transpose(ps2[:], h2t[:, 16:18], idb_sb[:])
                    nc.vector.tensor_copy(
                        fsd2[0:2, jt2 * 128:(jt2 + 1) * 128], ps2[0:2, :]
                    )
                nc.gpsimd.collective_compute(
                    "AllGather",
                    OP.bypass,
                    replica_groups=[list(range(NCORES))],
                    ins=[h2loc[:].opt()],
                    outs=[h2all_d[:].opt()],
                )
                dma_chunked(h2all_sb, h2all_d[:], 18, batch=True)
                for q4 in range(4):
                    nc.sync.dma_start(
                        f2T[1:2, q4 * 1024:(q4 + 1) * 1024], onesN_d[:]
                    )
                # srcpat2 rows: [ones; .8*fs2 (local, pre-scaled via W2a)]
                nc.vector.memset(srcpat2[0:1, :], 1.0)
                nc.sync.dma_start(srcpat2[1:2, :], fsd2[0:1, :])
                # layer-2 exp(.2 f_dst2) columns
                nc.scalar.activation(
                    vd2_sb[:, 0:NCH], h2all_sb[:, 17:NCH * 18:18],
                    AF.Exp, scale=ALPHA,
                )
                # h2e = [h2 (16) | zeros (16) | ones] per chunk: the ones
                # column puts Z2 at PSUM partition 32 (32-aligned for reads)
                nc.vector.memset(h2e_sb[:], 0.0)
                nc.vector.tensor_copy(
                    h2e_sb[:].rearrange("p (c o) -> p c o", o=33)[:, :, 0:16],
                    h2all_sb[:].rearrange("p (c o) -> p c o", o=18)[:, :, 0:16],
                )
                nc.vector.memset(
                    h2e_sb[:].rearrange("p (c o) -> p c o", o=33)[:, :, 32:33],
                    1.0,
                )

            # =============================================================
            # Phase 4: layer-2 attention + ELU + log_softmax
            # =============================================================
            with (
                tc.tile_pool(name="acc2", bufs=1, space="PSUM") as acc2,
                tc.tile_pool(name="pe2", bufs=2, space="PSUM") as e2pool,
                tc.tile_pool(name="sc2", bufs=3) as spool2,
                tc.tile_pool(name="fin", bufs=2) as fpool,
                tc.tile_pool(name="pfin", bufs=1, space="PSUM") as pfp2,
                tc.tile_pool(name="pc", bufs=2, space="PSUM") as pcp,
            ):
                po2 = acc2.tile([64, R], FP32, tag="o2")
                for c in range(NCH):
                    adj_c = adjT_sb[:, c * R:(c + 1) * R]
                    # f2T fd2-row chunk from h2all col 17 (interleaved so
                    # chunk 0's scores don't wait on all 32 transposes)
                    pcol = pcp.tile([1, 128], BF16, tag="pcol")
                    nc.tensor.transpose(
                        pcol[:], h2all_sb[:, c * 18 + 17:c * 18 + 18],
                        idb_sb[:],
                    )
                    nc.vector.tensor_copy(
                        f2T[0:1, c * 128:(c + 1) * 128], pcol[:]
                    )
                    pe2 = e2pool.tile([128, R], FP32, tag="pe2")
                    nc.tensor.matmul(
                        pe2[:], f2T[:, c * 128:(c + 1) * 128], srcpat2[:],
                        start=True, stop=False,
                    )
                    nc.tensor.matmul(
                        pe2[:], idBIG_sb[:], adj_c,
                        start=False, stop=True,
                    )
                    t1 = spool2.tile([128, R], BF16, tag="t1b")
                    nc.scalar.activation(
                        t1[:], pe2[:], AF.Exp, bias=bigneg[:, 0:1]
                    )
                    pt2 = spool2.tile([128, R], BF16, tag="pt2")
                    nc.vector.scalar_tensor_tensor(
                        pt2[:], adj_c, vd2_sb[:, c:c + 1], t1[:],
                        op0=OP.mult, op1=OP.max,
                    )
                    nc.tensor.matmul(
                        po2[0:33, :], h2e_sb[:, c * 33:(c + 1) * 33], pt2[:],
                        start=c == 0, stop=c == NCH - 1,
                    )
                # approx-reciprocal misreads PSUM at a non-zero partition
                # base; stage the Z2 row to SBUF partition 0 first
                z2sb = fpool.tile([1, R], FP32, tag="z2sb")
                nc.vector.tensor_copy(z2sb[:], po2[32:33, :])
                zinv2 = fpool.tile([1, R], FP32, tag="zinv2")
                nc.vector.reciprocal_approx_fast(zinv2[:], z2sb[:])
                zinv2b = fpool.tile([1, R], BF16, tag="zinv2b")
                nc.vector.tensor_copy(zinv2b[:], zinv2[:])
                zb2_ps = pfp2.tile([16, R], FP32, tag="zb2")
                nc.tensor.matmul(
                    zb2_ps[:], onesb_sb[0:1, 0:16], zinv2b[:],
                    start=True, stop=True,
                )
                zb2 = fpool.tile([16, R], FP32, tag="zb2s")
                nc.vector.tensor_copy(zb2[:], zb2_ps[:])
                pre2 = fpool.tile([16, R], FP32, tag="pre2")
                nc.vector.tensor_mul(pre2[:], po2[0:16, :], zb2[:])
                r2 = fpool.tile([16, R], FP32, tag="r2")
                nc.scalar.activation(r2[:], pre2[:], AF.Relu, scale=-1.0)
                t2e = fpool.tile([16, R], FP32, tag="t2e")
                nc.scalar.activation(t2e[:], r2[:], AF.Exp, scale=-1.0)
                u2 = fpool.tile([16, R], FP32, tag="u2")
                nc.vector.tensor_scalar_add(u2[:], t2e[:], -1.0)
                elu2 = fpool.tile([16, R], FP32, tag="elu2")
                nc.vector.tensor_max(elu2[:], pre2[:], u2[:])
                # transpose to natural [i, o2] then log_softmax over free
                # dim; exps batched before one Ln (fewer ACT table switches)
                pns, nmxs = [], []
                s_all = fpool.tile([128, 4], FP32, tag="s_all")
                for it in range(R // 128):
                    pn = fpool.tile([128, 16], FP32, tag=f"pn{it}",
                                    name=f"pn{it}")
                    pnp = pfp2.tile([128, 16], FP32, tag="pn")
                    nc.tensor.transpose(
                        pnp[:], elu2[:, it * 128:(it + 1) * 128],
                        idf_sb[:],
                    )
                    nc.vector.tensor_copy(pn[:], pnp[:])
                    nmx = fpool.tile([128, 1], FP32, tag=f"nmx{it}",
                                     name=f"nmx{it}")
                    nc.vector.tensor_reduce(
                        nmx[:], pn[:], AX.X, OP.max, negate=True
                    )
                    ex = fpool.tile([128, 16], FP32, tag="ex")
                    nc.scalar.activation(
                        ex[:], pn[:], AF.Exp, bias=nmx[:, 0:1],
                        accum_out=s_all[:, it:it + 1],
                    )
                    pns.append(pn); nmxs.append(nmx)
                lg = fpool.tile([128, 4], FP32, tag="lg")
                nc.scalar.activation(lg[:], s_all[:], AF.Ln)
                for it in range(R // 128):
                    fin = fpool.tile([128, 16], FP32, tag="fin")
                    nc.vector.tensor_scalar(
                        fin[:], pns[it][:], nmxs[it][:, 0:1], lg[:, it:it + 1],
                        op0=OP.add, op1=OP.subtract,
                    )
                    nc.sync.dma_start(out_d[it * 128:(it + 1) * 128, :], fin[:])

    nc.compile()
    return nc


def _get_nc():
    if "nc" not in _BUILD_CACHE:
        _BUILD_CACHE["nc"] = _build_nc()
    return _BUILD_CACHE["nc"]


def _prep_inputs(x, adj, W1, a_src1, a_dst1, W2, a_src2, a_dst2):
    bf16 = ml_dtypes.bfloat16
    f32 = np.float32
    x = np.asarray(x, f32)
    adj = np.asarray(adj, f32)
    W1 = np.asarray(W1, f32)
    W2 = np.asarray(W2, f32)
    a_src1 = np.asarray(a_src1, f32)
    a_dst1 = np.asarray(a_dst1, f32)
    a_src2 = np.asarray(a_src2, f32)
    a_dst2 = np.asarray(a_dst2, f32)

    W1f = np.ascontiguousarray(W1.reshape(F_IN, F1))
    # folded score vectors: f_src[h] = x @ (W1[:,h,:] @ a_src1[h])
    wsrc = np.stack([W1[:, h, :] @ a_src1[h] for h in range(H1)], axis=1)
    wdst = np.stack([W1[:, h, :] @ a_dst1[h] for h in range(H1)], axis=1)
    # pair p lives at partitions 32p..32p+2: [ones, fd_2p, fd_2p+1]
    wsd1 = np.zeros((F_IN, 128), f32)
    for p in range(4):
        wsd1[:, 32 * p + 1] = wdst[:, 2 * p]
        wsd1[:, 32 * p + 2] = wdst[:, 2 * p + 1]
    # .8*f_src columns placed so the transform emits rows at partition 32p
    wso1 = np.zeros((F_IN, 256), f32)
    for k in range(2):
        for p in range(4):
            wso1[:, 128 * k + 32 * p] = 0.8 * wsrc[:, 2 * p + k]
    W2f = np.ascontiguousarray(W2.reshape(F1, D2))
    W2a = np.zeros((F1, 18), f32)
    W2a[:, :D2] = W2f
    W2a[:, 16] = 0.8 * (W2f @ a_src2[0])
    W2a[:, 17] = W2f @ a_dst2[0]

    adjT_u8 = (adj.T > 0).astype(np.uint8)
    xb = x.astype(bf16)
    # xTa[p, (jt, fc, o)] = x[jt*128+o, fc*128+p]
    xTa = np.ascontiguousarray(
        xb.reshape(NCH, 128, NFC, 128).transpose(3, 0, 2, 1)
    ).reshape(128, NCH * NFC * 128)

    shared = {
        "W1f": W1f.astype(bf16),
        "wsd1": wsd1.astype(bf16),
        "wso1": wso1.astype(bf16),
        "wdn": wdst.astype(bf16),
        "W2a": W2a.astype(bf16),
        "xTa": xTa,
    }
    in_maps = []
    for c in range(NCORES):
        blkslice = slice(c * R, (c + 1) * R)
        m = dict(shared)
        m["adjU8"] = np.ascontiguousarray(adjT_u8[:, blkslice])
        m["xToa"] = np.ascontiguousarray(
            xTa[:, c * 4 * NFC * 128:(c + 1) * 4 * NFC * 128]
        )
        in_maps.append(m)
    return in_maps


def kernel(x, adj, W1, a_src1, a_dst1, W2, a_src2, a_dst2, _trace=False):
    from concourse.bass_utils import run_bass_kernel_spmd

    nc = _get_nc()
    in_maps = _prep_inputs(x, adj, W1, a_src1, a_dst1, W2, a_src2, a_dst2)
    res = run_bass_kernel_spmd(nc, in_maps, list(range(NCORES)), trace=_trace)
    out = np.concatenate(
        [np.asarray(res.results[c]["out"]) for c in range(NCORES)], axis=0
    )
    kernel.last_results = res
    return out.astype(np.float32)
